# revision 2
# baseline (speedup 1.0000x reference)
"""Trainium2 Bass kernel for BaseLayerWithLoRA:
    y = x @ W^T + b + (x @ lora_A^T) @ lora_B^T
  x [4,2048,4096] f32, W [4096,4096], b [4096], lora_A [16,4096], lora_B [4096,16]

Sharding: token-parallel across 8 cores (1024 tokens each, full O per core).
No collectives; LoRA computed per-core on its own token slice.

Per-core device program (all matmuls bf16, PE-roofline ~216ns per N=512 MM):
  warmup:  ~10 dummy matmuls on a memset tile keep the PE busy from body
           start (~7us) so the HAM clock-gate goes 8/8 before real work.
  phase A: arT[r=16, 1024] = lora_A @ x_c^T, col-tiled 4x: chunks kc=4q+j
           run concurrently in PE column groups j (tile_position=(0,32j)),
           partials land in PSUM partitions 32j..32j+16.
  interleave: phase A packs + the first 3 o-tiles' accumulation chains are
           issued chunk-synchronously so the PE has work the whole time x
           is streaming in.
  main:    outT[o-tile 128, tok 512] accumulated in PSUM over 32 K-chunks
           + 1 lora matmul (K=112: 4 copies of lora_B^T-slice interleaved
           with zero rows, contracting the 4 phase-A partials in one go).
           Bias fused into the PSUM->SBUF eviction (bf16 out).
  DMA:     scalar engine issues at/bias/W0-2 (so W0 lands ~11us instead of
           ~33us), sync issues the 32 x chunks then W3.., gpsimd issues
           output DMAs. Avoids the single-queue ~620ns/DMA issue serialization
           that starved the PE at startup.
Host does data layout only (transposes / tiling / packing), no arithmetic.
"""

import sys

if "/opt/trn_rl_repo" not in sys.path:
    sys.path.insert(0, "/opt/trn_rl_repo")

import numpy as np

B, S, I, O, R = 4, 2048, 4096, 4096, 16
NCORES = 8
NTOK = B * S                 # 8192 tokens
TPC = NTOK // NCORES         # 1024 tokens per core
NG = 4                       # phase-A column-tiling groups
LORA_K = 3 * 32 + R          # 112: lora contraction rows (zero-interleaved)


def build_nc(tpc=TPC, i_dim=I, o_dim=O, r=R, tok_tile=512, mm_dtype="bfloat16",
             n_dummy=10, n_pre_ot=3):
    import concourse.bacc as bacc
    import concourse.mybir as mybir
    import concourse.tile as tile

    KC = i_dim // 128        # contraction chunks
    OT = o_dim // 128        # output-row tiles
    TT = tpc // tok_tile     # token tiles
    WF = KC * 128 + 128      # per-o-tile weight blob free size (W chunk + lora)
    f32 = mybir.dt.float32
    f32r = getattr(mybir.dt, mm_dtype)

    nc = bacc.Bacc("TRN2", target_bir_lowering=False, debug=False)
    xt = nc.declare_dram_parameter("xt", [KC, 128, tpc], f32r, isOutput=False)
    wt = nc.declare_dram_parameter("wt", [OT, 128, WF], f32r, isOutput=False)
    at = nc.declare_dram_parameter("at", [128, KC, r], f32r, isOutput=False)
    bias = nc.declare_dram_parameter("bias", [128, OT], f32, isOutput=False)
    out = nc.declare_dram_parameter("out", [OT, 128, tpc], f32r, isOutput=True)

    with tile.TileContext(nc) as tc:
        with (
            tc.tile_pool(name="const", bufs=1) as constp,
            tc.tile_pool(name="xpool", bufs=KC) as xpool,
            tc.tile_pool(name="wpool", bufs=6) as wpool,
            tc.tile_pool(name="opool", bufs=3) as opool,
            tc.tile_pool(name="psum", bufs=6, space="PSUM") as psum_pool,
            tc.tile_pool(name="psum_ar", bufs=2, space="PSUM") as psum_ar_pool,
        ):
            # --- PE warmup: dummy matmuls with no DMA dependency ---
            dummy_w = constp.tile([128, 256], f32r, tag="dw", name="dummy_w")
            nc.gpsimd.memset(dummy_w[:], 0.0)
            dummy_ps = psum_pool.tile([128, tok_tile], f32, tag="chain",
                                      name="dummy_ps")
            for i in range(n_dummy):
                nc.tensor.matmul(
                    dummy_ps[:, 0:256], dummy_w[:, 0:128], dummy_w[:, 0:256],
                    start=True, stop=True,
                )

            # lora partials buffer: rows 32j..32j+16 hold group-j partials,
            # interleaved rows stay zero (memset) so the K=112 lora matmul
            # against the zero-interleaved lora_B^T blob sums the 4 groups.
            ar4 = constp.tile([LORA_K, tpc], f32r, tag="ar4", name="ar4")
            nc.gpsimd.memset(ar4[:], 0.0)

            # --- constants + first W tiles on the scalar queue ---
            at_sb = constp.tile([128, KC, r], f32r, tag="at", name="at_sb")
            nc.scalar.dma_start(at_sb[:], at[:])
            b_sb = constp.tile([128, OT], f32, tag="bias", name="b_sb")
            nc.scalar.dma_start(b_sb[:], bias[:])
            w_sbs = {}
            for ot in range(n_pre_ot):
                w_sbs[ot] = wpool.tile([128, WF], f32r, tag="w",
                                       name=f"w{ot}")
                nc.scalar.dma_start(w_sbs[ot][:], wt[ot])

            # --- x chunks on the sync queue ---
            xts = []
            for kc in range(KC):
                x_t = xpool.tile([128, tpc], f32r, tag="xchunk",
                                 name=f"xchunk{kc}")
                nc.sync.dma_start(x_t[:], xt[kc])
                xts.append(x_t)

            # --- interleaved phase A + first chains, chunk-synchronous ---
            pas = [
                psum_ar_pool.tile([128, tok_tile], f32, tag="pa",
                                  name=f"pa{h}")
                for h in range(TT)
            ]
            chain_ps = {}
            for ot in range(n_pre_ot):
                for h in range(TT):
                    chain_ps[(ot, h)] = psum_pool.tile(
                        [128, tok_tile], f32, tag="chain", name=f"cps{ot}_{h}"
                    )
            NQ = KC // NG
            for kq in range(NQ):
                for h in range(TT):
                    ts = slice(h * tok_tile, (h + 1) * tok_tile)
                    for j in range(NG):
                        kc = NG * kq + j
                        nc.tensor.matmul(
                            pas[h][32 * j : 32 * j + r, :],
                            at_sb[:, kc, :],
                            xts[kc][:, ts],
                            start=(kq == 0),
                            stop=(kq == NQ - 1),
                            tile_position=(0, 32 * j),
                        )
                for ot in range(n_pre_ot):
                    for h in range(TT):
                        ts = slice(h * tok_tile, (h + 1) * tok_tile)
                        for j in range(NG):
                            kc = NG * kq + j
                            nc.tensor.matmul(
                                chain_ps[(ot, h)][:],
                                w_sbs[ot][:, kc * 128 : (kc + 1) * 128],
                                xts[kc][:, ts],
                                start=(kc == 0),
                                stop=False,
                            )

            # --- phase A eviction: 4 lane-locked copies per token tile ---
            for h in range(TT):
                ts = slice(h * tok_tile, (h + 1) * tok_tile)
                for j in range(NG):
                    nc.vector.tensor_copy(
                        ar4[32 * j : 32 * j + r, ts],
                        pas[h][32 * j : 32 * j + r, :],
                    )

            def finish(ot, h, ps, w_sb):
                ts = slice(h * tok_tile, (h + 1) * tok_tile)
                nc.tensor.matmul(
                    ps[:],
                    w_sb[0:LORA_K, KC * 128 : KC * 128 + 128],
                    ar4[:, ts],
                    start=False,
                    stop=True,
                )
                o_sb = opool.tile([128, tok_tile], f32r, tag="o",
                                  name=f"o{ot}_{h}")
                nc.vector.tensor_scalar_add(o_sb[:], ps[:], b_sb[:, ot : ot + 1])
                nc.gpsimd.dma_start(out[ot, :, ts], o_sb[:])

            for ot in range(n_pre_ot):
                for h in range(TT):
                    finish(ot, h, chain_ps[(ot, h)], w_sbs[ot])

            # --- main loop: remaining o-tiles (W DMAs on sync, after x) ---
            for ot in range(n_pre_ot, OT):
                w_sb = wpool.tile([128, WF], f32r, tag="w", name=f"w{ot}")
                nc.sync.dma_start(w_sb[:], wt[ot])
                for h in range(TT):
                    ts = slice(h * tok_tile, (h + 1) * tok_tile)
                    ps = psum_pool.tile([128, tok_tile], f32, tag="chain",
                                        name=f"cps{ot}_{h}")
                    for kc in range(KC):
                        nc.tensor.matmul(
                            ps[:],
                            w_sb[:, kc * 128 : (kc + 1) * 128],
                            xts[kc][:, ts],
                            start=(kc == 0),
                            stop=False,
                        )
                    finish(ot, h, ps, w_sb)
    nc.compile()
    return nc


def prep_inputs(x, W, b, lora_A, lora_B, tpc=TPC, ncores=NCORES,
                mm_dtype="bfloat16"):
    """Host-side layout marshalling (layout + dtype cast only)."""
    import ml_dtypes

    np_mm = np.float32 if mm_dtype == "float32r" else np.dtype(ml_dtypes.bfloat16)
    i_dim, o_dim, r = W.shape[1], W.shape[0], lora_A.shape[0]
    ntok = tpc * ncores
    x = np.ascontiguousarray(x, dtype=np.float32).reshape(ntok, i_dim)
    W = np.ascontiguousarray(W, dtype=np.float32)
    b = np.ascontiguousarray(b, dtype=np.float32)
    lora_A = np.ascontiguousarray(lora_A, dtype=np.float32)
    lora_B = np.ascontiguousarray(lora_B, dtype=np.float32)

    KC, OT = i_dim // 128, o_dim // 128
    WF = KC * 128 + 128
    # wt blob per o-tile: [ki, kc*128+oo] = W[ot*128+oo, kc*128+ki],
    # last 128 cols: rows 32j..32j+r = lora_B^T slice for each group j
    # ([rr, oo] = lora_B[ot*128+oo, rr]), other rows zero.
    wtb = np.zeros((OT, 128, WF), dtype=np_mm)
    wtb[:, :, : KC * 128] = (
        W.reshape(OT, 128, KC, 128).transpose(0, 3, 2, 1).reshape(OT, 128, KC * 128)
    ).astype(np_mm)
    lbT = lora_B.reshape(OT, 128, r).transpose(0, 2, 1).astype(np_mm)
    for j in range(NG):
        wtb[:, 32 * j : 32 * j + r, KC * 128 :] = lbT
    # at[ki, kc, r] = lora_A[r, kc*128+ki]
    at = np.ascontiguousarray(
        lora_A.T.reshape(KC, 128, r).transpose(1, 0, 2).astype(np_mm)
    )
    # bias[p, ot] = b[ot*128+p]
    bias = np.ascontiguousarray(b.reshape(OT, 128).T)

    in_maps = []
    for c in range(ncores):
        xc = x[c * tpc : (c + 1) * tpc]  # [tpc, i_dim]
        # xt[kc, ki, t] = xc[t, kc*128+ki]
        xtc = np.ascontiguousarray(
            xc.reshape(tpc, KC, 128).transpose(1, 2, 0).astype(np_mm)
        )
        in_maps.append({"xt": xtc, "wt": wtb, "at": at, "bias": bias})
    return in_maps


def assemble_output(results):
    # each core: out[OT, 128, tpc] == y_c^T (bf16); tokens block-sharded
    outT = np.concatenate(
        [np.asarray(r["out"]) for r in results], axis=2
    )  # [OT,128,ntok]
    o_dim = outT.shape[0] * 128
    ntok = outT.shape[2]
    y = outT.reshape(o_dim, ntok).T.astype(np.float32)  # [ntok, o_dim]
    return np.ascontiguousarray(y)


def run(trace=False, trace_kwargs=None, mm_dtype="bfloat16", **inputs):
    from concourse.bass_utils import run_bass_kernel_spmd

    nc = build_nc(mm_dtype=mm_dtype)
    in_maps = prep_inputs(mm_dtype=mm_dtype, **inputs)
    res = run_bass_kernel_spmd(
        nc,
        in_maps,
        list(range(NCORES)),
        trace=trace,
        trace_kwargs=trace_kwargs or {},
    )
    return assemble_output(res.results).reshape(B, S, O), res


def kernel(**inputs):
    y, _ = run(trace=False, **inputs)
    return y


# revision 21
# speedup vs baseline: 1.2147x; 1.2147x over previous
"""Trainium2 Bass kernel for BaseLayerWithLoRA:
    y = x @ W^T + b + (x @ lora_A^T) @ lora_B^T
  x [4,2048,4096] f32, W [4096,4096], b [4096], lora_A [16,4096], lora_B [4096,16]

Sharding: token-parallel across 8 cores (1024 tokens each, full O per core).
No collectives; LoRA computed per-core on its own token slice.

Per-core device program (all matmuls bf16, PE-roofline ~216ns per N=512 MM):
  warmup:  ~10 dummy matmuls on a memset tile keep the PE busy from body
           start (~7us) so the HAM clock-gate goes 8/8 before real work.
  phase A: arT[r=16, 1024] = lora_A @ x_c^T, col-tiled 4x: chunks kc=4q+j
           run concurrently in PE column groups j (tile_position=(0,32j)),
           partials land in PSUM partitions 32j..32j+16.
  interleave: phase A packs + the first 3 o-tiles' accumulation chains are
           issued chunk-synchronously so the PE has work the whole time x
           is streaming in.
  main:    outT[o-tile 128, tok 512] accumulated in PSUM over 32 K-chunks
           + 1 lora matmul (K=112: 4 copies of lora_B^T-slice interleaved
           with zero rows, contracting the 4 phase-A partials in one go).
           Bias fused into the PSUM->SBUF eviction (bf16 out).
  DMA:     scalar HW queue issues at/bias/W0-2 halves (so W0's first half
           lands ~11us instead of ~33us) and later the output DMAs; sync HW
           queue issues the 32 x chunks then W3.. halves. Avoids both the
           single-queue ~620ns/DMA issue serialization that starved the PE
           at startup and the gpsimd software-DGE path (slow descriptors,
           ~7us final drain).
Host does data layout only (transposes / tiling / packing), no arithmetic.
"""

import sys

if "/opt/trn_rl_repo" not in sys.path:
    sys.path.insert(0, "/opt/trn_rl_repo")

import numpy as np

B, S, I, O, R = 4, 2048, 4096, 4096, 16
NCORES = 8
NTOK = B * S                 # 8192 tokens
TPC = NTOK // NCORES         # 1024 tokens per core
NG = 4                       # phase-A column-tiling groups
LORA_K = 3 * 32 + R          # 112: lora contraction rows (zero-interleaved)


def build_nc(tpc=TPC, i_dim=I, o_dim=O, r=R, tok_tile=512, mm_dtype="bfloat16",
             n_dummy=13, n_pre_ot=3):
    import concourse.bacc as bacc
    import concourse.mybir as mybir
    import concourse.tile as tile

    KC = i_dim // 128        # contraction chunks
    OT = o_dim // 128        # output-row tiles
    TT = tpc // tok_tile     # token tiles
    WF = KC * 128 + 128      # per-o-tile weight blob free size (W chunk + lora)
    f32 = mybir.dt.float32
    f32r = getattr(mybir.dt, mm_dtype)

    nc = bacc.Bacc("TRN2", target_bir_lowering=False, debug=False)
    xt = nc.declare_dram_parameter("xt", [KC, 128, tpc], f32r, isOutput=False)
    wt = nc.declare_dram_parameter("wt", [OT, 128, WF], f32r, isOutput=False)
    at = nc.declare_dram_parameter("at", [128, KC, r], f32r, isOutput=False)
    bias = nc.declare_dram_parameter("bias", [128, OT], f32, isOutput=False)
    out = nc.declare_dram_parameter("out", [OT, 128, tpc], f32r, isOutput=True)

    with tile.TileContext(nc) as tc:
        with (
            tc.tile_pool(name="const", bufs=1) as constp,
            tc.tile_pool(name="xpool", bufs=KC) as xpool,
            tc.tile_pool(name="wpool", bufs=6) as wpool,
            tc.tile_pool(name="opool", bufs=3) as opool,
            tc.tile_pool(name="psum", bufs=6, space="PSUM") as psum_pool,
            tc.tile_pool(name="psum_ar", bufs=2, space="PSUM") as psum_ar_pool,
        ):
            # --- PE warmup: dummy matmuls with no DMA dependency ---
            dummy_w = constp.tile([128, 256], f32r, tag="dw", name="dummy_w")
            nc.gpsimd.memset(dummy_w[:], 0.0)
            dummy_ps = psum_pool.tile([128, tok_tile], f32, tag="chain",
                                      name="dummy_ps")
            for i in range(n_dummy):
                nc.tensor.matmul(
                    dummy_ps[:, 0:256], dummy_w[:, 0:128], dummy_w[:, 0:256],
                    start=True, stop=True,
                )

            # lora partials buffer: rows 32j..32j+16 hold group-j partials,
            # interleaved rows stay zero (memset) so the K=112 lora matmul
            # against the zero-interleaved lora_B^T blob sums the 4 groups.
            ar4 = constp.tile([LORA_K, tpc], f32r, tag="ar4", name="ar4")
            nc.gpsimd.memset(ar4[:], 0.0)

            # --- constants + first W tiles on the scalar queue, half-split
            # so the first chains start ~13us; bias last (first eviction
            # does not read it until ~36us). ---
            at_sb = constp.tile([128, KC, r], f32r, tag="at", name="at_sb")
            nc.scalar.dma_start(at_sb[:], at[:])
            WH = (KC // 2) * 128  # half columns of the W blob
            w_sbs = {}
            for ot in range(n_pre_ot):
                w_sbs[ot] = wpool.tile([128, WF], f32r, tag="w",
                                       name=f"w{ot}")
                nc.scalar.dma_start(w_sbs[ot][:, :WH], wt[ot, :, :WH])
                nc.scalar.dma_start(w_sbs[ot][:, WH:], wt[ot, :, WH:])
            b_sb = constp.tile([128, OT], f32, tag="bias", name="b_sb")
            nc.scalar.dma_start(b_sb[:], bias[:])

            # --- x chunks on the sync queue ---
            xts = []
            for kc in range(KC):
                x_t = xpool.tile([128, tpc], f32r, tag="xchunk",
                                 name=f"xchunk{kc}")
                nc.sync.dma_start(x_t[:], xt[kc])
                xts.append(x_t)

            # --- interleaved phase A + first chains, chunk-synchronous ---
            pas = [
                psum_ar_pool.tile([128, tok_tile], f32, tag="pa",
                                  name=f"pa{h}")
                for h in range(TT)
            ]
            chain_ps = {}
            for ot in range(n_pre_ot):
                for h in range(TT):
                    chain_ps[(ot, h)] = psum_pool.tile(
                        [128, tok_tile], f32, tag="chain", name=f"cps{ot}_{h}"
                    )
            NQ = KC // NG
            for kq in range(NQ):
                for h in range(TT):
                    ts = slice(h * tok_tile, (h + 1) * tok_tile)
                    for j in range(NG):
                        kc = NG * kq + j
                        nc.tensor.matmul(
                            pas[h][32 * j : 32 * j + r, :],
                            at_sb[:, kc, :],
                            xts[kc][:, ts],
                            start=(kq == 0),
                            stop=(kq == NQ - 1),
                            tile_position=(0, 32 * j),
                        )
                for ot in range(n_pre_ot):
                    for h in range(TT):
                        ts = slice(h * tok_tile, (h + 1) * tok_tile)
                        for j in range(NG):
                            kc = NG * kq + j
                            nc.tensor.matmul(
                                chain_ps[(ot, h)][:],
                                w_sbs[ot][:, kc * 128 : (kc + 1) * 128],
                                xts[kc][:, ts],
                                start=(kc == 0),
                                stop=False,
                            )

            # --- phase A eviction: 4 lane-locked copies per token tile ---
            for h in range(TT):
                ts = slice(h * tok_tile, (h + 1) * tok_tile)
                for j in range(NG):
                    nc.vector.tensor_copy(
                        ar4[32 * j : 32 * j + r, ts],
                        pas[h][32 * j : 32 * j + r, :],
                    )

            def finish(ot, h, ps, w_sb, split=1):
                ts = slice(h * tok_tile, (h + 1) * tok_tile)
                nc.tensor.matmul(
                    ps[:],
                    w_sb[0:LORA_K, KC * 128 : KC * 128 + 128],
                    ar4[:, ts],
                    start=False,
                    stop=True,
                )
                o_sb = opool.tile([128, tok_tile], f32r, tag="o",
                                  name=f"o{ot}_{h}")
                qw = tok_tile // split
                for q in range(split):
                    qo = slice(q * qw, (q + 1) * qw)
                    qg = slice(h * tok_tile + q * qw, h * tok_tile + (q + 1) * qw)
                    nc.vector.tensor_scalar_add(
                        o_sb[:, qo], ps[:, qo], b_sb[:, ot : ot + 1]
                    )
                    nc.scalar.dma_start(out[ot, :, qg], o_sb[:, qo])

            for ot in range(n_pre_ot):
                for h in range(TT):
                    finish(ot, h, chain_ps[(ot, h)], w_sbs[ot])

            # --- main loop: remaining o-tiles (W DMAs on sync, after x) ---
            for ot in range(n_pre_ot, OT):
                w_sb = wpool.tile([128, WF], f32r, tag="w", name=f"w{ot}")
                nc.sync.dma_start(w_sb[:, :WH], wt[ot, :, :WH])
                nc.sync.dma_start(w_sb[:, WH:], wt[ot, :, WH:])
                for h in range(TT):
                    ts = slice(h * tok_tile, (h + 1) * tok_tile)
                    ps = psum_pool.tile([128, tok_tile], f32, tag="chain",
                                        name=f"cps{ot}_{h}")
                    for kc in range(KC):
                        nc.tensor.matmul(
                            ps[:],
                            w_sb[:, kc * 128 : (kc + 1) * 128],
                            xts[kc][:, ts],
                            start=(kc == 0),
                            stop=False,
                        )
                    last = ot == OT - 1 and h == TT - 1
                    finish(ot, h, ps, w_sb, split=2 if last else 1)
    nc.compile()
    return nc


def prep_inputs(x, W, b, lora_A, lora_B, tpc=TPC, ncores=NCORES,
                mm_dtype="bfloat16"):
    """Host-side layout marshalling (layout + dtype cast only)."""
    import ml_dtypes

    np_mm = np.float32 if mm_dtype == "float32r" else np.dtype(ml_dtypes.bfloat16)
    i_dim, o_dim, r = W.shape[1], W.shape[0], lora_A.shape[0]
    ntok = tpc * ncores
    x = np.ascontiguousarray(x, dtype=np.float32).reshape(ntok, i_dim)
    W = np.ascontiguousarray(W, dtype=np.float32)
    b = np.ascontiguousarray(b, dtype=np.float32)
    lora_A = np.ascontiguousarray(lora_A, dtype=np.float32)
    lora_B = np.ascontiguousarray(lora_B, dtype=np.float32)

    KC, OT = i_dim // 128, o_dim // 128
    WF = KC * 128 + 128
    # wt blob per o-tile: [ki, kc*128+oo] = W[ot*128+oo, kc*128+ki],
    # last 128 cols: rows 32j..32j+r = lora_B^T slice for each group j
    # ([rr, oo] = lora_B[ot*128+oo, rr]), other rows zero.
    wtb = np.zeros((OT, 128, WF), dtype=np_mm)
    wtb[:, :, : KC * 128] = (
        W.reshape(OT, 128, KC, 128).transpose(0, 3, 2, 1).reshape(OT, 128, KC * 128)
    ).astype(np_mm)
    lbT = lora_B.reshape(OT, 128, r).transpose(0, 2, 1).astype(np_mm)
    for j in range(NG):
        wtb[:, 32 * j : 32 * j + r, KC * 128 :] = lbT
    # at[ki, kc, r] = lora_A[r, kc*128+ki]
    at = np.ascontiguousarray(
        lora_A.T.reshape(KC, 128, r).transpose(1, 0, 2).astype(np_mm)
    )
    # bias[p, ot] = b[ot*128+p]
    bias = np.ascontiguousarray(b.reshape(OT, 128).T)

    in_maps = []
    for c in range(ncores):
        xc = x[c * tpc : (c + 1) * tpc]  # [tpc, i_dim]
        # xt[kc, ki, t] = xc[t, kc*128+ki]
        xtc = np.ascontiguousarray(
            xc.reshape(tpc, KC, 128).transpose(1, 2, 0).astype(np_mm)
        )
        in_maps.append({"xt": xtc, "wt": wtb, "at": at, "bias": bias})
    return in_maps


def assemble_output(results):
    # each core: out[OT, 128, tpc] == y_c^T (bf16); tokens block-sharded
    outT = np.concatenate(
        [np.asarray(r["out"]) for r in results], axis=2
    )  # [OT,128,ntok]
    o_dim = outT.shape[0] * 128
    ntok = outT.shape[2]
    y = outT.reshape(o_dim, ntok).T.astype(np.float32)  # [ntok, o_dim]
    return np.ascontiguousarray(y)


def run(trace=False, trace_kwargs=None, mm_dtype="bfloat16", **inputs):
    from concourse.bass_utils import run_bass_kernel_spmd

    nc = build_nc(mm_dtype=mm_dtype)
    in_maps = prep_inputs(mm_dtype=mm_dtype, **inputs)
    res = run_bass_kernel_spmd(
        nc,
        in_maps,
        list(range(NCORES)),
        trace=trace,
        trace_kwargs=trace_kwargs or {},
    )
    return assemble_output(res.results).reshape(B, S, O), res


def kernel(**inputs):
    y, _ = run(trace=False, **inputs)
    return y


# revision 25
# speedup vs baseline: 1.2148x; 1.0001x over previous
"""Trainium2 Bass kernel for BaseLayerWithLoRA:
    y = x @ W^T + b + (x @ lora_A^T) @ lora_B^T
  x [4,2048,4096] f32, W [4096,4096], b [4096], lora_A [16,4096], lora_B [4096,16]

Sharding: token-parallel across 8 cores (1024 tokens each, full O per core).
No collectives; LoRA computed per-core on its own token slice.

Per-core device program (all matmuls bf16, PE-roofline ~216ns per N=512 MM):
  warmup:  ~10 dummy matmuls on a memset tile keep the PE busy from body
           start (~7us) so the HAM clock-gate goes 8/8 before real work.
  phase A: arT[r=16, 1024] = lora_A @ x_c^T, col-tiled 4x: chunks kc=4q+j
           run concurrently in PE column groups j (tile_position=(0,32j)),
           partials land in PSUM partitions 32j..32j+16.
  interleave: phase A packs + the first 3 o-tiles' accumulation chains are
           issued chunk-synchronously so the PE has work the whole time x
           is streaming in.
  main:    outT[o-tile 128, tok 512] accumulated in PSUM over 32 K-chunks
           + 1 lora matmul (K=112: 4 copies of lora_B^T-slice interleaved
           with zero rows, contracting the 4 phase-A partials in one go).
           Bias fused into the PSUM->SBUF eviction (bf16 out).
  DMA:     scalar HW queue issues at/bias/W0-2 halves (so W0's first half
           lands ~11us instead of ~33us) and later the output DMAs; sync HW
           queue issues the 32 x chunks then W3.. halves. Avoids both the
           single-queue ~620ns/DMA issue serialization that starved the PE
           at startup and the gpsimd software-DGE path (slow descriptors,
           ~7us final drain).
Host does data layout only (transposes / tiling / packing), no arithmetic.
"""

import sys

if "/opt/trn_rl_repo" not in sys.path:
    sys.path.insert(0, "/opt/trn_rl_repo")

import numpy as np

B, S, I, O, R = 4, 2048, 4096, 4096, 16
NCORES = 8
NTOK = B * S                 # 8192 tokens
TPC = NTOK // NCORES         # 1024 tokens per core
NG = 4                       # phase-A column-tiling groups
LORA_K = 3 * 32 + R          # 112: lora contraction rows (zero-interleaved)


def build_nc(tpc=TPC, i_dim=I, o_dim=O, r=R, tok_tile=512, mm_dtype="bfloat16",
             n_dummy=13, n_pre_ot=3):
    import concourse.bacc as bacc
    import concourse.mybir as mybir
    import concourse.tile as tile

    KC = i_dim // 128        # contraction chunks
    OT = o_dim // 128        # output-row tiles
    TT = tpc // tok_tile     # token tiles
    WF = KC * 128 + 128      # per-o-tile weight blob free size (W chunk + lora)
    f32 = mybir.dt.float32
    f32r = getattr(mybir.dt, mm_dtype)

    nc = bacc.Bacc("TRN2", target_bir_lowering=False, debug=False)
    xt = nc.declare_dram_parameter("xt", [KC, 128, tpc], f32r, isOutput=False)
    wt = nc.declare_dram_parameter("wt", [OT, 128, WF], f32r, isOutput=False)
    at = nc.declare_dram_parameter("at", [128, KC, r], f32r, isOutput=False)
    bias = nc.declare_dram_parameter("bias", [128, OT], f32, isOutput=False)
    out = nc.declare_dram_parameter("out", [OT, 128, tpc], f32r, isOutput=True)

    with tile.TileContext(nc) as tc:
        with (
            tc.tile_pool(name="const", bufs=1) as constp,
            tc.tile_pool(name="xpool", bufs=KC) as xpool,
            tc.tile_pool(name="wpool", bufs=6) as wpool,
            tc.tile_pool(name="opool", bufs=3) as opool,
            tc.tile_pool(name="psum", bufs=6, space="PSUM") as psum_pool,
            tc.tile_pool(name="psum_ar", bufs=2, space="PSUM") as psum_ar_pool,
        ):
            # --- PE warmup: dummy matmuls with no DMA dependency ---
            dummy_w = constp.tile([128, 256], f32r, tag="dw", name="dummy_w")
            nc.gpsimd.memset(dummy_w[:], 0.0)
            dummy_ps = psum_pool.tile([128, tok_tile], f32, tag="chain",
                                      name="dummy_ps")
            for i in range(n_dummy):
                nc.tensor.matmul(
                    dummy_ps[:, 0:256], dummy_w[:, 0:128], dummy_w[:, 0:256],
                    start=True, stop=True,
                )

            # lora partials buffer: rows 32j..32j+16 hold group-j partials,
            # interleaved rows stay zero (memset) so the K=112 lora matmul
            # against the zero-interleaved lora_B^T blob sums the 4 groups.
            ar4 = constp.tile([LORA_K, tpc], f32r, tag="ar4", name="ar4")
            nc.gpsimd.memset(ar4[:], 0.0)

            # --- constants + first W tiles on the scalar queue, half-split
            # so the first chains start ~13us; bias last (first eviction
            # does not read it until ~36us). ---
            at_sb = constp.tile([128, KC, r], f32r, tag="at", name="at_sb")
            nc.scalar.dma_start(at_sb[:], at[:])
            WH = (KC // 2) * 128  # half columns of the W blob
            w_sbs = {}
            for ot in range(n_pre_ot):
                w_sbs[ot] = wpool.tile([128, WF], f32r, tag="w",
                                       name=f"w{ot}")
                nc.scalar.dma_start(w_sbs[ot][:, :WH], wt[ot, :, :WH])
                nc.scalar.dma_start(w_sbs[ot][:, WH:], wt[ot, :, WH:])
            b_sb = constp.tile([128, OT], f32, tag="bias", name="b_sb")
            nc.scalar.dma_start(b_sb[:], bias[:])

            # --- x chunks on the sync queue ---
            xts = []
            for kc in range(KC):
                x_t = xpool.tile([128, tpc], f32r, tag="xchunk",
                                 name=f"xchunk{kc}")
                nc.sync.dma_start(x_t[:], xt[kc])
                xts.append(x_t)

            # --- interleaved phase A + first chains, chunk-synchronous ---
            pas = [
                psum_ar_pool.tile([128, tok_tile], f32, tag="pa",
                                  name=f"pa{h}")
                for h in range(TT)
            ]
            chain_ps = {}
            for ot in range(n_pre_ot):
                for h in range(TT):
                    chain_ps[(ot, h)] = psum_pool.tile(
                        [128, tok_tile], f32, tag="chain", name=f"cps{ot}_{h}"
                    )
            NQ = KC // NG
            for kq in range(NQ):
                for h in range(TT):
                    ts = slice(h * tok_tile, (h + 1) * tok_tile)
                    for j in range(NG):
                        kc = NG * kq + j
                        nc.tensor.matmul(
                            pas[h][32 * j : 32 * j + r, :],
                            at_sb[:, kc, :],
                            xts[kc][:, ts],
                            start=(kq == 0),
                            stop=(kq == NQ - 1),
                            tile_position=(0, 32 * j),
                        )
                for ot in range(n_pre_ot):
                    for h in range(TT):
                        ts = slice(h * tok_tile, (h + 1) * tok_tile)
                        for j in range(NG):
                            kc = NG * kq + j
                            nc.tensor.matmul(
                                chain_ps[(ot, h)][:],
                                w_sbs[ot][:, kc * 128 : (kc + 1) * 128],
                                xts[kc][:, ts],
                                start=(kc == 0),
                                stop=False,
                            )

            # --- phase A eviction: 4 lane-locked copies per token tile ---
            for h in range(TT):
                ts = slice(h * tok_tile, (h + 1) * tok_tile)
                for j in range(NG):
                    nc.vector.tensor_copy(
                        ar4[32 * j : 32 * j + r, ts],
                        pas[h][32 * j : 32 * j + r, :],
                    )

            def finish(ot, h, ps, w_sb, split=1):
                ts = slice(h * tok_tile, (h + 1) * tok_tile)
                nc.tensor.matmul(
                    ps[:],
                    w_sb[0:LORA_K, KC * 128 : KC * 128 + 128],
                    ar4[:, ts],
                    start=False,
                    stop=True,
                )
                o_sb = opool.tile([128, tok_tile], f32r, tag="o",
                                  name=f"o{ot}_{h}")
                qw = tok_tile // split
                for q in range(split):
                    qo = slice(q * qw, (q + 1) * qw)
                    qg = slice(h * tok_tile + q * qw, h * tok_tile + (q + 1) * qw)
                    nc.vector.tensor_scalar_add(
                        o_sb[:, qo], ps[:, qo], b_sb[:, ot : ot + 1]
                    )
                    nc.scalar.dma_start(out[ot, :, qg], o_sb[:, qo])

            for ot in range(n_pre_ot):
                for h in range(TT):
                    finish(ot, h, chain_ps[(ot, h)], w_sbs[ot])

            # --- main loop: remaining o-tiles (W DMAs on sync, after x) ---
            for ot in range(n_pre_ot, OT):
                w_sb = wpool.tile([128, WF], f32r, tag="w", name=f"w{ot}")
                nc.sync.dma_start(w_sb[:, :WH], wt[ot, :, :WH])
                nc.sync.dma_start(w_sb[:, WH:], wt[ot, :, WH:])
                for h in range(TT):
                    ts = slice(h * tok_tile, (h + 1) * tok_tile)
                    ps = psum_pool.tile([128, tok_tile], f32, tag="chain",
                                        name=f"cps{ot}_{h}")
                    for kc in range(KC):
                        nc.tensor.matmul(
                            ps[:],
                            w_sb[:, kc * 128 : (kc + 1) * 128],
                            xts[kc][:, ts],
                            start=(kc == 0),
                            stop=False,
                        )
                    last = ot == OT - 1 and h == TT - 1
                    finish(ot, h, ps, w_sb, split=2 if last else 1)
    nc.compile()
    return nc


def prep_inputs(x, W, b, lora_A, lora_B, tpc=TPC, ncores=NCORES,
                mm_dtype="bfloat16"):
    """Host-side layout marshalling (layout + dtype cast only)."""
    import ml_dtypes

    np_mm = np.float32 if mm_dtype == "float32r" else np.dtype(ml_dtypes.bfloat16)
    i_dim, o_dim, r = W.shape[1], W.shape[0], lora_A.shape[0]
    ntok = tpc * ncores
    x = np.ascontiguousarray(x, dtype=np.float32).reshape(ntok, i_dim)
    W = np.ascontiguousarray(W, dtype=np.float32)
    b = np.ascontiguousarray(b, dtype=np.float32)
    lora_A = np.ascontiguousarray(lora_A, dtype=np.float32)
    lora_B = np.ascontiguousarray(lora_B, dtype=np.float32)

    KC, OT = i_dim // 128, o_dim // 128
    WF = KC * 128 + 128
    # wt blob per o-tile: [ki, kc*128+oo] = W[ot*128+oo, kc*128+ki],
    # last 128 cols: rows 32j..32j+r = lora_B^T slice for each group j
    # ([rr, oo] = lora_B[ot*128+oo, rr]), other rows zero.
    wtb = np.zeros((OT, 128, WF), dtype=np_mm)
    wtb[:, :, : KC * 128] = (
        W.reshape(OT, 128, KC, 128).transpose(0, 3, 2, 1).reshape(OT, 128, KC * 128)
    ).astype(np_mm)
    lbT = lora_B.reshape(OT, 128, r).transpose(0, 2, 1).astype(np_mm)
    for j in range(NG):
        wtb[:, 32 * j : 32 * j + r, KC * 128 :] = lbT
    # at[ki, kc, r] = lora_A[r, kc*128+ki]
    at = np.ascontiguousarray(
        lora_A.T.reshape(KC, 128, r).transpose(1, 0, 2).astype(np_mm)
    )
    # bias[p, ot] = b[ot*128+p]
    bias = np.ascontiguousarray(b.reshape(OT, 128).T)

    in_maps = []
    for c in range(ncores):
        xc = x[c * tpc : (c + 1) * tpc]  # [tpc, i_dim]
        # xt[kc, ki, t] = xc[t, kc*128+ki]
        xtc = np.ascontiguousarray(
            xc.reshape(tpc, KC, 128).transpose(1, 2, 0).astype(np_mm)
        )
        in_maps.append({"xt": xtc, "wt": wtb, "at": at, "bias": bias})
    return in_maps


def assemble_output(results):
    # each core: out[OT, 128, tpc] == y_c^T (bf16); tokens block-sharded
    outT = np.concatenate(
        [np.asarray(r["out"]) for r in results], axis=2
    )  # [OT,128,ntok]
    o_dim = outT.shape[0] * 128
    ntok = outT.shape[2]
    y = outT.reshape(o_dim, ntok).T.astype(np.float32)  # [ntok, o_dim]
    return np.ascontiguousarray(y)


def run(trace=False, trace_kwargs=None, mm_dtype="bfloat16", **inputs):
    from concourse.bass_utils import run_bass_kernel_spmd

    nc = build_nc(mm_dtype=mm_dtype)
    in_maps = prep_inputs(mm_dtype=mm_dtype, **inputs)
    res = run_bass_kernel_spmd(
        nc,
        in_maps,
        list(range(NCORES)),
        trace=trace,
        trace_kwargs=trace_kwargs or {},
    )
    return assemble_output(res.results).reshape(B, S, O), res


def kernel(**inputs):
    y, _ = run(trace=False, **inputs)
    return y


# revision 27
# speedup vs baseline: 1.2162x; 1.0011x over previous
"""Trainium2 Bass kernel for BaseLayerWithLoRA:
    y = x @ W^T + b + (x @ lora_A^T) @ lora_B^T
  x [4,2048,4096] f32, W [4096,4096], b [4096], lora_A [16,4096], lora_B [4096,16]

Sharding: token-parallel across 8 cores (1024 tokens each, full O per core).
No collectives; LoRA computed per-core on its own token slice.

Per-core device program (all matmuls bf16, PE-roofline ~216ns per N=512 MM):
  warmup:  ~10 dummy matmuls on a memset tile keep the PE busy from body
           start (~7us) so the HAM clock-gate goes 8/8 before real work.
  phase A: arT[r=16, 1024] = lora_A @ x_c^T, col-tiled 4x: chunks kc=4q+j
           run concurrently in PE column groups j (tile_position=(0,32j)),
           partials land in PSUM partitions 32j..32j+16.
  interleave: phase A packs + the first 3 o-tiles' accumulation chains are
           issued chunk-synchronously so the PE has work the whole time x
           is streaming in.
  main:    outT[o-tile 128, tok 512] accumulated in PSUM over 32 K-chunks
           + 1 lora matmul (K=112: 4 copies of lora_B^T-slice interleaved
           with zero rows, contracting the 4 phase-A partials in one go).
           Bias fused into the PSUM->SBUF eviction (bf16 out).
  DMA:     scalar HW queue issues at/bias/W0-2 halves (so W0's first half
           lands ~11us instead of ~33us) and later the output DMAs; sync HW
           queue issues the 32 x chunks then W3.. halves. Avoids both the
           single-queue ~620ns/DMA issue serialization that starved the PE
           at startup and the gpsimd software-DGE path (slow descriptors,
           ~7us final drain).
Host does data layout only (transposes / tiling / packing), no arithmetic.
"""

import sys

if "/opt/trn_rl_repo" not in sys.path:
    sys.path.insert(0, "/opt/trn_rl_repo")

import numpy as np

B, S, I, O, R = 4, 2048, 4096, 4096, 16
NCORES = 8
NTOK = B * S                 # 8192 tokens
TPC = NTOK // NCORES         # 1024 tokens per core
NG = 4                       # phase-A column-tiling groups
LORA_K = 3 * 32 + R          # 112: lora contraction rows (zero-interleaved)


def build_nc(tpc=TPC, i_dim=I, o_dim=O, r=R, tok_tile=512, mm_dtype="bfloat16",
             n_dummy=13, n_pre_ot=3):
    import concourse.bacc as bacc
    import concourse.mybir as mybir
    import concourse.tile as tile

    KC = i_dim // 128        # contraction chunks
    OT = o_dim // 128        # output-row tiles
    TT = tpc // tok_tile     # token tiles
    WF = KC * 128 + 128      # per-o-tile weight blob free size (W chunk + lora)
    f32 = mybir.dt.float32
    f32r = getattr(mybir.dt, mm_dtype)

    nc = bacc.Bacc("TRN2", target_bir_lowering=False, debug=False)
    xt = nc.declare_dram_parameter("xt", [KC, 128, tpc], f32r, isOutput=False)
    wt = nc.declare_dram_parameter("wt", [OT, 128, WF], f32r, isOutput=False)
    at = nc.declare_dram_parameter("at", [128, KC, r], f32r, isOutput=False)
    bias = nc.declare_dram_parameter("bias", [128, OT], f32, isOutput=False)
    out = nc.declare_dram_parameter("out", [OT, 128, tpc], f32r, isOutput=True)

    with tile.TileContext(nc) as tc:
        with (
            tc.tile_pool(name="const", bufs=1) as constp,
            tc.tile_pool(name="xpool", bufs=KC) as xpool,
            tc.tile_pool(name="wpool", bufs=6) as wpool,
            tc.tile_pool(name="opool", bufs=3) as opool,
            tc.tile_pool(name="psum", bufs=6, space="PSUM") as psum_pool,
            tc.tile_pool(name="psum_ar", bufs=2, space="PSUM") as psum_ar_pool,
        ):
            # --- PE warmup: dummy matmuls with no DMA dependency ---
            dummy_w = constp.tile([128, 256], f32r, tag="dw", name="dummy_w")
            nc.gpsimd.memset(dummy_w[:], 0.0)
            dummy_ps = psum_pool.tile([128, tok_tile], f32, tag="chain",
                                      name="dummy_ps")
            for i in range(n_dummy):
                nc.tensor.matmul(
                    dummy_ps[:, 0:256], dummy_w[:, 0:128], dummy_w[:, 0:256],
                    start=True, stop=True,
                )

            # lora partials buffer: rows 32j..32j+16 hold group-j partials,
            # interleaved rows stay zero (memset) so the K=112 lora matmul
            # against the zero-interleaved lora_B^T blob sums the 4 groups.
            ar4 = constp.tile([LORA_K, tpc], f32r, tag="ar4", name="ar4")
            nc.gpsimd.memset(ar4[:], 0.0)

            # --- constants + first W tiles on the scalar queue, half-split
            # so the first chains start ~13us; bias last (first eviction
            # does not read it until ~36us). ---
            at_sb = constp.tile([128, KC, r], f32r, tag="at", name="at_sb")
            nc.scalar.dma_start(at_sb[:], at[:])
            WH = (KC // 2) * 128  # half columns of the W blob
            w_sbs = {}
            for ot in range(n_pre_ot):
                w_sbs[ot] = wpool.tile([128, WF], f32r, tag="w",
                                       name=f"w{ot}")
                nc.scalar.dma_start(w_sbs[ot][:, :WH], wt[ot, :, :WH])
                nc.scalar.dma_start(w_sbs[ot][:, WH:], wt[ot, :, WH:])
            b_sb = constp.tile([128, OT], f32, tag="bias", name="b_sb")
            nc.scalar.dma_start(b_sb[:], bias[:])

            # --- x chunks on the sync queue ---
            xts = []
            for kc in range(KC):
                x_t = xpool.tile([128, tpc], f32r, tag="xchunk",
                                 name=f"xchunk{kc}")
                nc.sync.dma_start(x_t[:], xt[kc])
                xts.append(x_t)

            # --- interleaved phase A + first chains, chunk-synchronous ---
            pas = [
                psum_ar_pool.tile([128, tok_tile], f32, tag="pa",
                                  name=f"pa{h}")
                for h in range(TT)
            ]
            chain_ps = {}
            for ot in range(n_pre_ot):
                for h in range(TT):
                    chain_ps[(ot, h)] = psum_pool.tile(
                        [128, tok_tile], f32, tag="chain", name=f"cps{ot}_{h}"
                    )
            NQ = KC // NG
            for kq in range(NQ):
                for h in range(TT):
                    ts = slice(h * tok_tile, (h + 1) * tok_tile)
                    for j in range(NG):
                        kc = NG * kq + j
                        nc.tensor.matmul(
                            pas[h][32 * j : 32 * j + r, :],
                            at_sb[:, kc, :],
                            xts[kc][:, ts],
                            start=(kq == 0),
                            stop=(kq == NQ - 1),
                            tile_position=(0, 32 * j),
                        )
                for ot in range(n_pre_ot):
                    for h in range(TT):
                        ts = slice(h * tok_tile, (h + 1) * tok_tile)
                        for j in range(NG):
                            kc = NG * kq + j
                            nc.tensor.matmul(
                                chain_ps[(ot, h)][:],
                                w_sbs[ot][:, kc * 128 : (kc + 1) * 128],
                                xts[kc][:, ts],
                                start=(kc == 0),
                                stop=False,
                            )

            # --- phase A eviction: 4 lane-locked copies per token tile ---
            for h in range(TT):
                ts = slice(h * tok_tile, (h + 1) * tok_tile)
                for j in range(NG):
                    nc.vector.tensor_copy(
                        ar4[32 * j : 32 * j + r, ts],
                        pas[h][32 * j : 32 * j + r, :],
                    )

            def finish(ot, h, ps, w_sb, split=1):
                ts = slice(h * tok_tile, (h + 1) * tok_tile)
                nc.tensor.matmul(
                    ps[:],
                    w_sb[0:LORA_K, KC * 128 : KC * 128 + 128],
                    ar4[:, ts],
                    start=False,
                    stop=True,
                )
                o_sb = opool.tile([128, tok_tile], f32r, tag="o",
                                  name=f"o{ot}_{h}")
                qw = tok_tile // split
                for q in range(split):
                    qo = slice(q * qw, (q + 1) * qw)
                    qg = slice(h * tok_tile + q * qw, h * tok_tile + (q + 1) * qw)
                    nc.vector.tensor_scalar_add(
                        o_sb[:, qo], ps[:, qo], b_sb[:, ot : ot + 1]
                    )
                    nc.scalar.dma_start(out[ot, :, qg], o_sb[:, qo])

            for ot in range(n_pre_ot):
                for h in range(TT):
                    finish(ot, h, chain_ps[(ot, h)], w_sbs[ot])

            # --- main loop: remaining o-tiles (W DMAs on sync, after x) ---
            for ot in range(n_pre_ot, OT):
                w_sb = wpool.tile([128, WF], f32r, tag="w", name=f"w{ot}")
                nc.sync.dma_start(w_sb[:, :WH], wt[ot, :, :WH])
                nc.sync.dma_start(w_sb[:, WH:], wt[ot, :, WH:])
                for h in range(TT):
                    ts = slice(h * tok_tile, (h + 1) * tok_tile)
                    ps = psum_pool.tile([128, tok_tile], f32, tag="chain",
                                        name=f"cps{ot}_{h}")
                    for kc in range(KC):
                        nc.tensor.matmul(
                            ps[:],
                            w_sb[:, kc * 128 : (kc + 1) * 128],
                            xts[kc][:, ts],
                            start=(kc == 0),
                            stop=False,
                        )
                    last = ot == OT - 1 and h == TT - 1
                    finish(ot, h, ps, w_sb, split=2 if last else 1)
    nc.compile()
    return nc


def prep_inputs(x, W, b, lora_A, lora_B, tpc=TPC, ncores=NCORES,
                mm_dtype="bfloat16"):
    """Host-side layout marshalling (layout + dtype cast only)."""
    import ml_dtypes

    np_mm = np.float32 if mm_dtype == "float32r" else np.dtype(ml_dtypes.bfloat16)
    i_dim, o_dim, r = W.shape[1], W.shape[0], lora_A.shape[0]
    ntok = tpc * ncores
    x = np.ascontiguousarray(x, dtype=np.float32).reshape(ntok, i_dim)
    W = np.ascontiguousarray(W, dtype=np.float32)
    b = np.ascontiguousarray(b, dtype=np.float32)
    lora_A = np.ascontiguousarray(lora_A, dtype=np.float32)
    lora_B = np.ascontiguousarray(lora_B, dtype=np.float32)

    KC, OT = i_dim // 128, o_dim // 128
    WF = KC * 128 + 128
    # wt blob per o-tile: [ki, kc*128+oo] = W[ot*128+oo, kc*128+ki],
    # last 128 cols: rows 32j..32j+r = lora_B^T slice for each group j
    # ([rr, oo] = lora_B[ot*128+oo, rr]), other rows zero.
    wtb = np.zeros((OT, 128, WF), dtype=np_mm)
    wtb[:, :, : KC * 128] = (
        W.reshape(OT, 128, KC, 128).transpose(0, 3, 2, 1).reshape(OT, 128, KC * 128)
    ).astype(np_mm)
    lbT = lora_B.reshape(OT, 128, r).transpose(0, 2, 1).astype(np_mm)
    for j in range(NG):
        wtb[:, 32 * j : 32 * j + r, KC * 128 :] = lbT
    # at[ki, kc, r] = lora_A[r, kc*128+ki]
    at = np.ascontiguousarray(
        lora_A.T.reshape(KC, 128, r).transpose(1, 0, 2).astype(np_mm)
    )
    # bias[p, ot] = b[ot*128+p]
    bias = np.ascontiguousarray(b.reshape(OT, 128).T)

    in_maps = []
    for c in range(ncores):
        xc = x[c * tpc : (c + 1) * tpc]  # [tpc, i_dim]
        # xt[kc, ki, t] = xc[t, kc*128+ki]
        xtc = np.ascontiguousarray(
            xc.reshape(tpc, KC, 128).transpose(1, 2, 0).astype(np_mm)
        )
        in_maps.append({"xt": xtc, "wt": wtb, "at": at, "bias": bias})
    return in_maps


def assemble_output(results):
    # each core: out[OT, 128, tpc] == y_c^T (bf16); tokens block-sharded
    outT = np.concatenate(
        [np.asarray(r["out"]) for r in results], axis=2
    )  # [OT,128,ntok]
    o_dim = outT.shape[0] * 128
    ntok = outT.shape[2]
    y = outT.reshape(o_dim, ntok).T.astype(np.float32)  # [ntok, o_dim]
    return np.ascontiguousarray(y)


def run(trace=False, trace_kwargs=None, mm_dtype="bfloat16", **inputs):
    from concourse.bass_utils import run_bass_kernel_spmd

    nc = build_nc(mm_dtype=mm_dtype)
    in_maps = prep_inputs(mm_dtype=mm_dtype, **inputs)
    res = run_bass_kernel_spmd(
        nc,
        in_maps,
        list(range(NCORES)),
        trace=trace,
        trace_kwargs=trace_kwargs or {},
    )
    return assemble_output(res.results).reshape(B, S, O), res


def kernel(**inputs):
    y, _ = run(trace=False, **inputs)
    return y
